# revision 18
# baseline (speedup 1.0000x reference)
"""BitLinear (ternary-weight linear with int8-absmax-quantized activations) on 8 trn2 cores.

Math (reference, GROUPS=1): with mean = mean(weight), sign = sign(weight - mean),
beta = mean(|sign|), the reference computes round(127/max|x| * x) @ (sign*beta).T / beta.
For this problem beta cancels (and equals 1.0 when no weight element equals the mean
exactly), so the output is exactly quant @ sign.T -- pure integer arithmetic:
quant in [-127, 127], sign in {-1, 0, 1}, dot products < 2^24.  Both operands are
therefore exactly representable in bf16 and fp32 PSUM accumulation is exact, so the
bf16 TensorE path reproduces the reference bit-for-bit up to a handful of
rounding-boundary ULPs in the quantization step.

Sharding: tensor-parallel 2D grid, 4-way over out_features x 2-way over tokens.
Each core gets x.T for its token slice (fp32, quantized on device via the
round-to-nearest-even magic-number trick on VectorE) and w.T for its out_features
slice (fp32, ternarized on device via ScalarE Sign).  The global scalars
(activation scale and weight mean) are computed on host with the same jnp ops the
reference uses (bit-identical on the same backend) and baked into the kernel as
immediates.  Matmul: lhsT = quant tile [128k, 128t], moving rhs = ternary weights
[128k, 512o], PSUM [128t, 512o], accumulated over k with groups of t-tiles so the
weight DMA/ternarize streams behind the TensorE.
"""

import numpy as np

TOKENS = 8192
K = 4096  # in_features (contraction dim)
OUT = 4096  # out_features
O_SHARDS = 4  # shards along out_features
T_SHARDS = 2  # shards along tokens
O = OUT // O_SHARDS  # 1024 out_features per core
T = TOKENS // T_SHARDS  # 4096 tokens per core
QB = 127  # int8 absmax bound, as in the reference
MAGIC = 12582912.0  # 1.5 * 2**23: fp32 (x + MAGIC) - MAGIC == rint(x) for |x| < 2**22
N_CORES = 8

_REPO = "/opt/trn_rl_repo"
last_results = None  # BassKernelResults of the most recent kernel() call


def _import_concourse():
    import sys

    if _REPO not in sys.path:
        sys.path.insert(0, _REPO)


def emit_body(tc, xt_ap, wt_ap, out_ap, scale, neg_mean, t_dim, o_dim, k_dim, group=2,
              qbufs=4, xbufs=2, wsbufs=3, obufs=3, psbufs=8, w_first=False,
              chunks=2, chunks0=None, group0=None, fill_kk=0):
    """Emit the per-core program.

    xt_ap:  DRAM [t_dim//128, k_dim, 128] fp32   (x.T, blocked by 128-token tiles)
    wt_ap:  DRAM [k_dim, o_dim] fp32             (w.T slice for this core)
    out_ap: DRAM [t_dim, o_dim] fp32             (out[t, o] for this core's slices)
    """
    _import_concourse()
    import concourse.mybir as mybir

    dt = mybir.dt
    alu = mybir.AluOpType
    nc = tc.nc

    P = 128
    MMF = 512  # matmul moving free dim == one fp32 PSUM bank
    TT = t_dim // P  # token tiles
    KK = k_dim // P  # contraction tiles
    NO = o_dim // MMF  # out_features chunks
    assert t_dim % P == 0 and k_dim % P == 0 and o_dim % MMF == 0
    group = min(group, TT)

    if qbufs is None:
        qbufs = group + 2
    with (
        tc.tile_pool(name="w3pool", bufs=1) as w3pool,
        tc.tile_pool(name="wstage", bufs=wsbufs) as wstage,
        tc.tile_pool(name="xstage", bufs=xbufs) as xstage,
        tc.tile_pool(name="qpool", bufs=qbufs) as qpool,
        tc.tile_pool(name="opool", bufs=obufs) as opool,
        tc.tile_pool(name="pspool", bufs=psbufs, space="PSUM") as pspool,
    ):
        # Per-partition scalar constants for ScalarE activation bias operands.
        consts = w3pool.tile([P, 2], dt.float32, tag="consts")
        nc.vector.memset(consts[:, 0:1], -MAGIC)
        nc.vector.memset(consts[:, 1:2], neg_mean)
        neg_magic_ap = consts[:, 0:1]
        neg_mean_ap = consts[:, 1:2]

        # Resident ternarized weights for this core: [128, KK, o_dim] bf16.
        w3 = w3pool.tile([P, KK, o_dim], dt.bfloat16)

        CH = min(chunks, KK)
        CH0 = min(chunks0 or chunks, KK)  # first group may chunk finer
        assert KK % CH == 0 and KK % CH0 == 0

        def alloc_q(tt):
            xs = xstage.tile([P, KK, P], dt.float32, tag="xs", name="xs")
            qt = qpool.tile([P, KK, P], dt.bfloat16, tag="q", name="q")
            return xs, qt

        def make_q_chunk(tt, xs, qt, k0, k1):
            # Load x.T rows for k-tiles [k0, k1) as [128, k1-k0, 128].
            sl = slice(k0, k1)
            nc.sync.dma_start(
                xs[:, sl, :],
                xt_ap[tt, k0 * P : k1 * P, :].rearrange("(kk p) t -> p kk t", p=P),
            )
            # rint(scale*x) via the magic trick; the +MAGIC add rounds to integer.
            nc.vector.tensor_scalar(
                xs[:, sl, :], xs[:, sl, :], scale, MAGIC, alu.mult, alu.add
            )
            # Subtract MAGIC back out (exact; result is an integer in [-127, 127]).
            nc.scalar.activation(
                qt[:, sl, :],
                xs[:, sl, :],
                mybir.ActivationFunctionType.Identity,
                bias=neg_magic_ap,
            )

        def make_w3(kk):
            ws = wstage.tile([P, o_dim], dt.float32, tag="ws", name="ws")
            nc.sync.dma_start(ws[:], wt_ap[kk * P : (kk + 1) * P, :])
            nc.scalar.sign(w3[:, kk, :], ws[:], bias=neg_mean_ap)

        if w_first:
            for kk in range(KK):
                make_w3(kk)

        sizes = []
        left = TT
        if group0 is not None and group0 != group and group0 <= TT:
            sizes.append(group0)
            left -= group0
        while left > 0:
            sizes.append(min(group, left))
            left -= sizes[-1]

        first_group = True
        g0 = 0
        for gsize in sizes:
            grp = range(g0, g0 + gsize)
            g0 += gsize
            qs = {tt: alloc_q(tt) for tt in grp}
            ps = {
                (tt, no): pspool.tile([P, MMF], dt.float32, tag="ps", name="ps")
                for tt in grp
                for no in range(NO)
            }
            # All chunks emitted up front, in chunk-major order: the
            # DMA -> DVE -> ACT chain pipelines at chunk granularity, so the
            # first matmul only waits for chunk 0 of each tile.  In the first
            # group the W strips are interleaved in consumption order so the
            # serialized DMA stream matches what the PE needs next.
            ch = CH0 if first_group else CH
            kkc = KK // ch
            wins = [(c * kkc, (c + 1) * kkc) for c in range(ch)]
            if first_group and fill_kk and fill_kk < wins[0][1]:
                # Shrink the very first window so the first matmul's
                # DMA -> DVE -> ACT chain is short.
                wins = [(0, fill_kk), (fill_kk, wins[0][1])] + wins[1:]
            if first_group and not w_first:
                make_w3(0)
            for k0, k1 in wins:
                for tt in grp:
                    make_q_chunk(tt, *qs[tt], k0, k1)
                if first_group and not w_first:
                    for kk in range(max(1, k0), k1):
                        make_w3(kk)
            for kk in range(KK):
                last = kk == KK - 1
                for tt in grp:
                    ob = opool.tile([P, o_dim], dt.float32, tag="ob", name="ob") if last else None
                    for no in range(NO):
                        nc.tensor.matmul(
                            ps[(tt, no)],
                            qs[tt][1][:, kk, :],
                            w3[:, kk, no * MMF : (no + 1) * MMF],
                            start=(kk == 0),
                            stop=last,
                        )
                        if last:
                            nc.vector.tensor_copy(
                                ob[:, no * MMF : (no + 1) * MMF], ps[(tt, no)]
                            )
                    if last:
                        nc.sync.dma_start(out_ap[tt * P : (tt + 1) * P, :], ob[:])
            first_group = False


def _build_nc(scale, neg_mean, t_dim=T, o_dim=O, k_dim=K):
    _import_concourse()
    import concourse.bacc as bacc
    import concourse.mybir as mybir
    import concourse.tile as tile

    dt = mybir.dt
    nc = bacc.Bacc("TRN2", target_bir_lowering=False, debug=False)
    xt = nc.dram_tensor(
        "xt", [t_dim // 128, k_dim, 128], dt.float32, kind="ExternalInput"
    ).ap()
    wt = nc.dram_tensor("wt", [k_dim, o_dim], dt.float32, kind="ExternalInput").ap()
    out = nc.dram_tensor("out", [t_dim, o_dim], dt.float32, kind="ExternalOutput").ap()
    with tile.TileContext(nc) as tc:
        emit_body(tc, xt, wt, out, scale, neg_mean, t_dim, o_dim, k_dim)
    nc.compile()
    return nc


def host_scalars(x, w):
    """scale and mean, computed with the same jnp ops (and backend) the reference uses."""
    import jax.numpy as jnp

    wg = jnp.asarray(w).reshape(1, -1)
    mean = np.asarray(jnp.mean(wg, axis=1, keepdims=True)).astype(np.float32)[0, 0]
    scale = np.asarray(QB / jnp.max(jnp.abs(jnp.asarray(x)))).astype(np.float32)[()]
    return float(scale), float(mean)


def shard_inputs(x, w):
    """Per-core input maps for the 4 (out_features) x 2 (tokens) grid."""
    xt_shards = []
    for t_idx in range(T_SHARDS):
        xs = x[t_idx * T : (t_idx + 1) * T, :]
        # [T, K] -> [TT, K, 128] so each 128-token tile of x.T is contiguous
        xt = np.ascontiguousarray(xs.reshape(T // 128, 128, K).transpose(0, 2, 1))
        xt_shards.append(xt)
    wt_shards = []
    for o_idx in range(O_SHARDS):
        wt = np.ascontiguousarray(w[o_idx * O : (o_idx + 1) * O, :].T)
        wt_shards.append(wt)
    return [
        {"xt": xt_shards[c % T_SHARDS], "wt": wt_shards[c // T_SHARDS]}
        for c in range(N_CORES)
    ]


def kernel(input, weight, **run_kwargs):
    _import_concourse()
    from concourse import bass_utils

    x = np.ascontiguousarray(np.asarray(input, dtype=np.float32))
    w = np.ascontiguousarray(np.asarray(weight, dtype=np.float32))

    scale, mean = host_scalars(x, w)
    nc = _build_nc(scale, -mean)
    in_maps = shard_inputs(x, w)

    res = bass_utils.run_bass_kernel_spmd(
        nc, in_maps, core_ids=list(range(N_CORES)), **run_kwargs
    )
    global last_results
    last_results = res

    out = np.empty((TOKENS, OUT), dtype=np.float32)
    for c in range(N_CORES):
        o_idx, t_idx = c // T_SHARDS, c % T_SHARDS
        out[t_idx * T : (t_idx + 1) * T, o_idx * O : (o_idx + 1) * O] = res.results[c][
            "out"
        ]
    return out


# revision 25
# speedup vs baseline: 1.0025x; 1.0025x over previous
"""BitLinear (ternary-weight linear with int8-absmax-quantized activations) on 8 trn2 cores.

Math (reference, GROUPS=1): with mean = mean(weight), sign = sign(weight - mean),
beta = mean(|sign|), the reference computes round(127/max|x| * x) @ (sign*beta).T / beta.
For this problem beta cancels (and equals 1.0 when no weight element equals the mean
exactly), so the output is exactly quant @ sign.T -- pure integer arithmetic:
quant in [-127, 127], sign in {-1, 0, 1}, dot products < 2^24.  Both operands are
therefore exactly representable in bf16 and fp32 PSUM accumulation is exact, so the
bf16 TensorE path reproduces the reference bit-for-bit up to a handful of
rounding-boundary ULPs in the quantization step.

Sharding: tensor-parallel 2D grid, 4-way over out_features x 2-way over tokens.
Each core gets x.T for its token slice (fp32, quantized on device via the
round-to-nearest-even magic-number trick on VectorE) and w.T for its out_features
slice (fp32, ternarized on device via ScalarE Sign).  The global scalars
(activation scale and weight mean) are computed on host with the same jnp ops the
reference uses (bit-identical on the same backend) and baked into the kernel as
immediates.  Matmul: lhsT = quant tile [128k, 128t], moving rhs = ternary weights
[128k, 512o], PSUM [128t, 512o], accumulated over k with groups of t-tiles so the
weight DMA/ternarize streams behind the TensorE.
"""

import numpy as np

TOKENS = 8192
K = 4096  # in_features (contraction dim)
OUT = 4096  # out_features
O_SHARDS = 4  # shards along out_features
T_SHARDS = 2  # shards along tokens
O = OUT // O_SHARDS  # 1024 out_features per core
T = TOKENS // T_SHARDS  # 4096 tokens per core
QB = 127  # int8 absmax bound, as in the reference
MAGIC = 12582912.0  # 1.5 * 2**23: fp32 (x + MAGIC) - MAGIC == rint(x) for |x| < 2**22
N_CORES = 8

_REPO = "/opt/trn_rl_repo"
last_results = None  # BassKernelResults of the most recent kernel() call


def _import_concourse():
    import sys

    if _REPO not in sys.path:
        sys.path.insert(0, _REPO)


def emit_body(tc, xt_ap, wt_ap, out_ap, scale, neg_mean, t_dim, o_dim, k_dim, group=2,
              qbufs=4, xbufs=2, wsbufs=3, obufs=4, psbufs=8, w_first=False,
              chunks=2, chunks0=None, group0=None, fill_kk=0, warmup=12):
    """Emit the per-core program.

    xt_ap:  DRAM [t_dim//128, k_dim, 128] fp32   (x.T, blocked by 128-token tiles)
    wt_ap:  DRAM [k_dim, o_dim] fp32             (w.T slice for this core)
    out_ap: DRAM [t_dim, o_dim] fp32             (out[t, o] for this core's slices)
    """
    _import_concourse()
    import concourse.mybir as mybir

    dt = mybir.dt
    alu = mybir.AluOpType
    nc = tc.nc

    P = 128
    MMF = 512  # matmul moving free dim == one fp32 PSUM bank
    TT = t_dim // P  # token tiles
    KK = k_dim // P  # contraction tiles
    NO = o_dim // MMF  # out_features chunks
    assert t_dim % P == 0 and k_dim % P == 0 and o_dim % MMF == 0
    group = min(group, TT)

    if qbufs is None:
        qbufs = group + 2
    with (
        tc.tile_pool(name="w3pool", bufs=1) as w3pool,
        tc.tile_pool(name="wstage", bufs=wsbufs) as wstage,
        tc.tile_pool(name="xstage", bufs=xbufs) as xstage,
        tc.tile_pool(name="qpool", bufs=qbufs) as qpool,
        tc.tile_pool(name="opool", bufs=obufs) as opool,
        tc.tile_pool(name="pspool", bufs=psbufs, space="PSUM") as pspool,
    ):
        # Per-partition scalar constants for ScalarE activation bias operands.
        consts = w3pool.tile([P, 2], dt.float32, tag="consts")
        nc.vector.memset(consts[:, 0:1], -MAGIC)
        nc.vector.memset(consts[:, 1:2], neg_mean)
        neg_magic_ap = consts[:, 0:1]
        neg_mean_ap = consts[:, 1:2]

        # Resident ternarized weights for this core: [128, KK, o_dim] bf16.
        w3 = w3pool.tile([P, KK, o_dim], dt.bfloat16)

        if warmup:
            # Junk matmuls issued while the input pipeline fills: the PE would
            # otherwise idle here, and sustained activity releases the HAM
            # clock gate so the first real matmuls run at full rate.  The
            # banks they touch are cleared by the real start=True matmuls.
            wj = w3pool.tile([P, P], dt.bfloat16, tag="warm_l")
            mj = w3pool.tile([P, MMF], dt.bfloat16, tag="warm_r")
            nc.vector.memset(wj[:], 0.0)
            nc.vector.memset(mj[:], 0.0)
            pj = pspool.tile([P, MMF], dt.float32, tag="ps", name="ps_warm")
            for _ in range(warmup):
                nc.tensor.matmul(pj, wj[:], mj[:], start=True, stop=True)

        CH = min(chunks, KK)
        CH0 = min(chunks0 or chunks, KK)  # first group may chunk finer
        assert KK % CH == 0 and KK % CH0 == 0

        def alloc_q(tt):
            xs = xstage.tile([P, KK, P], dt.float32, tag="xs", name="xs")
            qt = qpool.tile([P, KK, P], dt.bfloat16, tag="q", name="q")
            return xs, qt

        def make_q_chunk(tt, xs, qt, k0, k1, pass2_dve=False):
            # Load x.T rows for k-tiles [k0, k1) as [128, k1-k0, 128].
            sl = slice(k0, k1)
            nc.sync.dma_start(
                xs[:, sl, :],
                xt_ap[tt, k0 * P : k1 * P, :].rearrange("(kk p) t -> p kk t", p=P),
            )
            # rint(scale*x) via the magic trick; the +MAGIC add rounds to integer.
            nc.vector.tensor_scalar(
                xs[:, sl, :], xs[:, sl, :], scale, MAGIC, alu.mult, alu.add
            )
            # Subtract MAGIC back out (exact; result is an integer in [-127, 127]).
            # While the first group's W strips keep ScalarE busy with Sign,
            # route this step to the vector engine instead.
            if pass2_dve:
                nc.vector.tensor_scalar_add(qt[:, sl, :], xs[:, sl, :], -MAGIC)
            else:
                nc.scalar.activation(
                    qt[:, sl, :],
                    xs[:, sl, :],
                    mybir.ActivationFunctionType.Identity,
                    bias=neg_magic_ap,
                )

        def make_w3(kk):
            ws = wstage.tile([P, o_dim], dt.float32, tag="ws", name="ws")
            nc.sync.dma_start(ws[:], wt_ap[kk * P : (kk + 1) * P, :])
            nc.scalar.sign(w3[:, kk, :], ws[:], bias=neg_mean_ap)

        if w_first:
            for kk in range(KK):
                make_w3(kk)

        sizes = []
        left = TT
        if group0 is not None and group0 != group and group0 <= TT:
            sizes.append(group0)
            left -= group0
        while left > 0:
            sizes.append(min(group, left))
            left -= sizes[-1]

        first_group = True
        g0 = 0
        for gsize in sizes:
            grp = range(g0, g0 + gsize)
            g0 += gsize
            qs = {tt: alloc_q(tt) for tt in grp}
            ps = {
                (tt, no): pspool.tile([P, MMF], dt.float32, tag="ps", name="ps")
                for tt in grp
                for no in range(NO)
            }
            # All chunks emitted up front, in chunk-major order: the
            # DMA -> DVE -> ACT chain pipelines at chunk granularity, so the
            # first matmul only waits for chunk 0 of each tile.  In the first
            # group the W strips are interleaved in consumption order so the
            # serialized DMA stream matches what the PE needs next.
            ch = CH0 if first_group else CH
            kkc = KK // ch
            wins = [(c * kkc, (c + 1) * kkc) for c in range(ch)]
            if first_group and fill_kk and fill_kk < wins[0][1]:
                # Shrink the very first window so the first matmul's
                # DMA -> DVE -> ACT chain is short.
                wins = [(0, fill_kk), (fill_kk, wins[0][1])] + wins[1:]
            if first_group and not w_first:
                make_w3(0)
            for k0, k1 in wins:
                for tt in grp:
                    make_q_chunk(tt, *qs[tt], k0, k1)
                if first_group and not w_first:
                    for kk in range(max(1, k0), k1):
                        make_w3(kk)
            def drain(tt, no):
                # Drain + store per 512-column chunk; alternate DVE/ScalarE so
                # back-to-back drains run on two engines in parallel.
                ob = opool.tile([P, MMF], dt.float32, tag="ob", name="ob")
                if (tt + no) % 2 == 0:
                    nc.vector.tensor_copy(ob[:], ps[(tt, no)])
                else:
                    nc.scalar.copy(ob[:], ps[(tt, no)])
                nc.sync.dma_start(
                    out_ap[tt * P : (tt + 1) * P, no * MMF : (no + 1) * MMF],
                    ob[:],
                )

            if g0 < TT:
                for kk in range(KK):
                    last = kk == KK - 1
                    for tt in grp:
                        for no in range(NO):
                            nc.tensor.matmul(
                                ps[(tt, no)],
                                qs[tt][1][:, kk, :],
                                w3[:, kk, no * MMF : (no + 1) * MMF],
                                start=(kk == 0),
                                stop=last,
                            )
                            if last:
                                drain(tt, no)
            else:
                # Final group: per-(tile, chunk) k-runs so every drain except
                # the very last overlaps remaining matmuls, shortening the
                # kernel's tail chain.
                for tt in grp:
                    for no in range(NO):
                        for kk in range(KK):
                            nc.tensor.matmul(
                                ps[(tt, no)],
                                qs[tt][1][:, kk, :],
                                w3[:, kk, no * MMF : (no + 1) * MMF],
                                start=(kk == 0),
                                stop=(kk == KK - 1),
                            )
                        drain(tt, no)
            first_group = False


def _build_nc(scale, neg_mean, t_dim=T, o_dim=O, k_dim=K):
    _import_concourse()
    import concourse.bacc as bacc
    import concourse.mybir as mybir
    import concourse.tile as tile

    dt = mybir.dt
    nc = bacc.Bacc("TRN2", target_bir_lowering=False, debug=False)
    xt = nc.dram_tensor(
        "xt", [t_dim // 128, k_dim, 128], dt.float32, kind="ExternalInput"
    ).ap()
    wt = nc.dram_tensor("wt", [k_dim, o_dim], dt.float32, kind="ExternalInput").ap()
    out = nc.dram_tensor("out", [t_dim, o_dim], dt.float32, kind="ExternalOutput").ap()
    with tile.TileContext(nc) as tc:
        emit_body(tc, xt, wt, out, scale, neg_mean, t_dim, o_dim, k_dim)
    nc.compile()
    return nc


def host_scalars(x, w):
    """scale and mean, computed with the same jnp ops (and backend) the reference uses."""
    import jax.numpy as jnp

    wg = jnp.asarray(w).reshape(1, -1)
    mean = np.asarray(jnp.mean(wg, axis=1, keepdims=True)).astype(np.float32)[0, 0]
    scale = np.asarray(QB / jnp.max(jnp.abs(jnp.asarray(x)))).astype(np.float32)[()]
    return float(scale), float(mean)


def shard_inputs(x, w):
    """Per-core input maps for the 4 (out_features) x 2 (tokens) grid."""
    xt_shards = []
    for t_idx in range(T_SHARDS):
        xs = x[t_idx * T : (t_idx + 1) * T, :]
        # [T, K] -> [TT, K, 128] so each 128-token tile of x.T is contiguous
        xt = np.ascontiguousarray(xs.reshape(T // 128, 128, K).transpose(0, 2, 1))
        xt_shards.append(xt)
    wt_shards = []
    for o_idx in range(O_SHARDS):
        wt = np.ascontiguousarray(w[o_idx * O : (o_idx + 1) * O, :].T)
        wt_shards.append(wt)
    return [
        {"xt": xt_shards[c % T_SHARDS], "wt": wt_shards[c // T_SHARDS]}
        for c in range(N_CORES)
    ]


def kernel(input, weight, **run_kwargs):
    _import_concourse()
    from concourse import bass_utils

    x = np.ascontiguousarray(np.asarray(input, dtype=np.float32))
    w = np.ascontiguousarray(np.asarray(weight, dtype=np.float32))

    scale, mean = host_scalars(x, w)
    nc = _build_nc(scale, -mean)
    in_maps = shard_inputs(x, w)

    res = bass_utils.run_bass_kernel_spmd(
        nc, in_maps, core_ids=list(range(N_CORES)), **run_kwargs
    )
    global last_results
    last_results = res

    out = np.empty((TOKENS, OUT), dtype=np.float32)
    for c in range(N_CORES):
        o_idx, t_idx = c // T_SHARDS, c % T_SHARDS
        out[t_idx * T : (t_idx + 1) * T, o_idx * O : (o_idx + 1) * O] = res.results[c][
            "out"
        ]
    return out
